# revision 10
# baseline (speedup 1.0000x reference)
"""Trainium2 Bass kernel for nn_ContrastiveLoss_dot (MISA contrastive loss).

Math (reference):
  M[i,j,r,w] = <im[j,r,:], s[i,w,:]>            # (B,B,R,W) matchmap
  M_max      = max_r M                          # (B,B,W)
  sims[i,j]  = sum_{w<n_i} M_max / n_i          # (B,B)
  scores     = sims.T                           # scores[img j, cap i]
  loss = sum over off-diag of relu(m + scores - rowdiag)
       + sum over off-diag of relu(m + scores - coldiag)

Sharding: data-parallel over captions. Captions are length-balanced across
the 8 cores (snake-deal on sorted n_i) and only the valid words of each
caption are packed into the per-core word axis.  Per core on device:
  OUT[w, (j,r)] = sT.T @ imT  (packed words x 4608 regions) as fp8-e4m3
  DoubleRow matmuls (2 k-tiles per instruction), then a segmented max over
  r (36 regions/image) -> M_max[w, j] in bf16 via DVE reduce_max.
M_max ships back to the host, which does the (exact) per-caption word sums,
the division by n_i, and the final (B,B)->scalar hinge loss in float64.

Schedule notes (from neuron-profile traces; ~48.5-49 us cool vs 52.2 us
for the previous stage-2-on-device + no-warmup kernel):
 * Timeline: NEFF preamble to first DMA trigger ~7.2 us (fixed), sT DMA
   completes ~12.8 us (the gate for real matmuls), PE then runs gapless
   ~33 us at ~213 ns per 504-col fp8 DoubleRow matmul (1 col/cycle at
   2.4 GHz -- the hw max), tail ~4.8 us (last reduce + out DMA cold
   latency ~2 us + NEFF end barrier).
 * The PE clock sits at the 1.2 GHz mid p-state until several us of busy
   time accumulate.  N_WARM dummy matmuls on a scratch tile keep the PE
   busy from preamble end (~7.3 us) while the input DMAs land, so real
   matmuls start near full clock (measured -1.25 us).  More warmup delays
   the real work past the sT arrival and burns throttle headroom (worse
   when the part is hot), so n_warm stays small.
 * im rides the sync-engine HW-DGE queue, sT + the M_max output ride the
   scalar-engine HW-DGE queue (the only two HW DGE queues on TRN2; both
   stripe over the same 16 shared DMA engines).
 * Measured queue characteristics that shaped this: first-DMA cold
   latency ~2-3.5 us per queue; the sync queue chains subsequent DMAs at
   ~0.3 us but the scalar queue needs ~2.3 us between DMAs.  Splitting
   sT or the output into multiple DMAs therefore LOSES time (each extra
   piece pays latency); the single 512 KB sT DMA is the optimum.
 * Chunk sizes 8/14 images (N=288/504 psum cols) are equivalent to
   uniform 13s within noise; LDWEIGHTS hides even under 120 ns matmuls.
 * <= 9 DMAs total: a 10th overflows the DMA semaphore pool and later
   triggers serialize on semaphore-epoch reuse (measured +6us).
"""

import sys

if "/opt/trn_rl_repo" not in sys.path:
    sys.path.insert(0, "/opt/trn_rl_repo")

import numpy as np

B, R, W, D = 128, 36, 60, 1024
N_CORES = 8
CAPS = B // N_CORES              # 16 captions per core
KT = D // 128                    # 8 contraction tiles
KP = KT // 2                     # 4 DoubleRow k-pair passes
MARGIN = 0.2

CONFIG = dict(
    im_split=(8, 28, 28, 28, 28, 8),   # images per supertile (sums to 128)
    st_groups=(4,),                    # wt tiles per sT DMA
    n_warm=10,                         # p-state warmup matmuls
    warm_cols=504,
    absorb=(),                         # supertiles needing a wait-absorber
)

_CACHE = {}


def _chunks_of(g):
    """Split a supertile of g images into psum-bank chunks (<=14 images)."""
    n = -(-g // 14)
    base, rem = divmod(g, n)
    return [base + (1 if i < rem else 0) for i in range(n)]


def _build_nc(wt_tiles, cfg):
    """Bass program for one core; word axis = wt_tiles*128 packed words."""
    import concourse.tile as tile
    from concourse import bacc, mybir

    fp8 = mybir.dt.float8e4
    bf16 = mybir.dt.bfloat16
    f32 = mybir.dt.float32
    DR = mybir.MatmulPerfMode.DoubleRow
    wpad = wt_tiles * 128
    im_split = list(cfg["im_split"])
    st_groups = list(cfg["st_groups"])
    assert sum(im_split) == B and sum(st_groups) == wt_tiles
    n_dma = len(im_split) + len(st_groups) + (2 if cfg.get("out_split") else 1)
    assert n_dma <= 9, f"{n_dma} DMAs overflows the semaphore pool"
    IM_LEN = KT * B * R  # flattened supertile-major im length per partition

    nc = bacc.Bacc("TRN2", target_bir_lowering=False, debug=False,
                   num_devices=1)

    # im_lin[p, off_g + k*cols_g + nn] = e4m3(im[j, r, k*128+p]),
    # nn = (j - j0_g)*R + r  -- supertile-major so each DMA is contiguous
    im_lin = nc.dram_tensor("im_lin", [128, IM_LEN], fp8,
                            kind="ExternalInput").ap()
    # sT8[p, (t*KT + k)*128 + m] = e4m3(sT[k*128 + p, t*128 + m])
    # (word-tile major so each ST_GROUPS slice is one contiguous DMA)
    sT8 = nc.dram_tensor("sT8", [128, wt_tiles * KT * 128], fp8,
                         kind="ExternalInput").ap()
    # out[p, t*B + j] = bf16 M_max for packed word t*128+p vs image j
    out = nc.dram_tensor("out", [128, wt_tiles * B], bf16,
                         kind="ExternalOutput").ap()

    with tile.TileContext(nc) as tc:
        with (
            tc.tile_pool(name="p_pool", bufs=1) as p_pool,
            tc.tile_pool(name="im_pool", bufs=1) as im_pool,
            tc.tile_pool(name="mx_pool", bufs=1) as mx_pool,
            tc.tile_pool(name="wm_pool", bufs=1) as wm_pool,
            tc.tile_pool(name="ps1", bufs=7, space="PSUM") as ps1_pool,
            tc.tile_pool(name="ps2", bufs=1, space="PSUM") as ps2_pool,
        ):
            # p-state warmup: gpsimd memsets a scratch tile right after the
            # preamble; dummy matmuls keep the PE busy (ramping its clock to
            # 2.4 GHz) while the real input DMAs are still in flight.
            warm_ps = None
            if cfg["n_warm"] or cfg["absorb"]:
                warm_ps = ps2_pool.tile([128, max(cfg["warm_cols"], 2)], f32)
            if cfg["n_warm"]:
                warm_sb = wm_pool.tile([128, 512], fp8)
                if cfg.get("warm_iota"):
                    # varied data draws more datapath power than zeros ->
                    # pushes the DVFS boost harder during warmup
                    nc.gpsimd.iota(
                        warm_sb[:], pattern=[[1, 512]], base=0,
                        channel_multiplier=1,
                        allow_small_or_imprecise_dtypes=True)
                else:
                    nc.gpsimd.memset(warm_sb[:], 0)
                for _ in range(cfg["n_warm"]):
                    nc.tensor.matmul(
                        warm_ps[:], warm_sb[:, 0:128],
                        warm_sb[:, 0:cfg["warm_cols"]],
                        start=True, stop=True)
                # tiny tail matmuls: keep the PE busy (clock pinned) with
                # fine granularity right up to the sT DMA arrival -- a gap
                # here drops the clock back to the 1.2 GHz p-state for the
                # first ~3 us of real matmuls.
                for _ in range(cfg.get("n_tail", 0)):
                    nc.tensor.matmul(
                        warm_ps[:, 0:cfg.get("tail_cols", 64)],
                        warm_sb[:, 0:128],
                        warm_sb[:, 0:cfg.get("tail_cols", 64)],
                        start=True, stop=True)

            # sT in word-tile groups on the scalar HW-DGE queue
            sT_sb = p_pool.tile([128, wt_tiles, KT, 128], fp8)
            t0 = 0
            for g in st_groups:
                nc.scalar.dma_start(
                    sT_sb[:, t0:t0 + g],
                    sT8[:, t0 * KT * 128:(t0 + g) * KT * 128].rearrange(
                        "p (t k m) -> p t k m", t=g, k=KT))
                t0 += g

            def sT_pair(kp, wt):
                return sT_sb[:, wt, 2 * kp:2 * kp + 2, :]

            mmax_sb = mx_pool.tile([128, wt_tiles, B], bf16)  # words x images

            # stream im once; fused segmented max per psum chunk
            img0 = 0
            off = 0
            ci = 0  # global chunk index (for absorber bookkeeping)
            for gi, G in enumerate(im_split):
                cols = G * R
                im_sb = im_pool.tile([128, KT, cols], fp8, tag=f"im{gi}")
                nc.sync.dma_start(
                    im_sb[:],
                    im_lin[:, off:off + KT * cols].rearrange(
                        "p (k n) -> p k n", k=KT))
                if gi in cfg["absorb"]:
                    # wait-absorber: takes the im-DMA wait on the PE queue
                    # (PE instructions have a single sync-wait slot)
                    nc.tensor.matmul(
                        warm_ps[:, 0:2],
                        im_sb[:, 0, 0:128], im_sb[:, 0, 0:2],
                        start=True, stop=True)
                c0 = 0
                for cn in _chunks_of(G):
                    for wt in range(wt_tiles):
                        ps = ps1_pool.tile([128, 14, R], f32, tag="ps")
                        for kp in range(KP):
                            nc.tensor.matmul(
                                ps[:, :cn, :],
                                sT_pair(kp, wt),
                                im_sb[:, 2 * kp:2 * kp + 2,
                                      c0 * R:(c0 + cn) * R],
                                start=(kp == 0),
                                stop=(kp == KP - 1),
                                perf_mode=DR,
                            )
                        nc.vector.reduce_max(
                            mmax_sb[:, wt, img0 + c0:img0 + c0 + cn],
                            ps[:, :cn, :],
                            axis=mybir.AxisListType.X,
                        )
                    c0 += cn
                    ci += 1
                img0 += G
                off += KT * cols

            # ship M_max; host does the per-caption word sums + hinge loss.
            # out_split=j cuts the output DMA at image column j so the bulk
            # ships while the last chunks still compute (subtile deps).
            out_r = out.rearrange("p (t b) -> p t b", t=wt_tiles)
            js = cfg.get("out_split", 0)
            if js:
                nc.scalar.dma_start(out_r[:, :, 0:js], mmax_sb[:, :, 0:js])
                nc.scalar.dma_start(out_r[:, :, js:B], mmax_sb[:, :, js:B])
            else:
                nc.scalar.dma_start(out_r, mmax_sb[:])

    nc.compile()
    return nc


def get_nc(wt_tiles, cfg=None):
    cfg = cfg or CONFIG
    key = ("nc", wt_tiles, tuple(sorted(
        (k, tuple(v) if isinstance(v, (list, tuple)) else v)
        for k, v in cfg.items())))
    if key not in _CACHE:
        _CACHE[key] = _build_nc(wt_tiles, cfg)
    return _CACHE[key]


def assign_captions(s_l):
    """Length-balanced snake assignment: caps_of_core[c] = 16 caption ids."""
    n = np.asarray(s_l).astype(np.int64)
    order = np.argsort(-n, kind="stable")  # longest first
    caps_of_core = [[] for _ in range(N_CORES)]
    for r in range(CAPS):
        chunk = order[r * N_CORES:(r + 1) * N_CORES]
        cores = range(N_CORES) if r % 2 == 0 else range(N_CORES - 1, -1, -1)
        for c, cap in zip(cores, chunk):
            caps_of_core[c].append(int(cap))
    return caps_of_core


def make_core_inputs(im, s, s_l, cfg=None):
    """Host-side shard prep. Returns (in_maps, caps_of_core, wt_tiles)."""
    import ml_dtypes

    cfg = cfg or CONFIG
    e4m3 = ml_dtypes.float8_e4m3fn
    im = np.ascontiguousarray(im, dtype=np.float32)
    s = np.ascontiguousarray(s, dtype=np.float32)
    n = np.asarray(s_l).astype(np.int64)

    caps_of_core = assign_captions(s_l)
    packed = [int(sum(n[i] for i in caps)) for caps in caps_of_core]
    wt_tiles = max(1, -(-max(packed) // 128))  # ceil to 128
    wpad = wt_tiles * 128

    # [p, k, n] with n = j*R + r
    imr = np.ascontiguousarray(
        im.reshape(B * R, KT, 128).transpose(2, 1, 0)).astype(e4m3)
    pieces = []
    n0 = 0
    for G in cfg["im_split"]:
        cols = G * R
        pieces.append(np.ascontiguousarray(
            imr[:, :, n0:n0 + cols]).reshape(128, KT * cols))
        n0 += cols
    im_lin = np.ascontiguousarray(np.concatenate(pieces, axis=1))

    in_maps = []
    for c in range(N_CORES):
        sT = np.zeros((D, wpad), dtype=np.float32)
        off = 0
        for cap in caps_of_core[c]:
            ni = int(n[cap])
            sT[:, off:off + ni] = s[cap, :ni, :].T
            off += ni
        # sT8[p, (t*KT + k)*128 + m] = sT[k*128 + p, t*128 + m]  (e4m3)
        sT8 = np.ascontiguousarray(
            sT.reshape(KT, 128, wt_tiles, 128).transpose(1, 2, 0, 3)
            .reshape(128, -1).astype(e4m3))
        in_maps.append({"im_lin": im_lin, "sT8": sT8})
    return in_maps, caps_of_core, wt_tiles


def loss_from_sims(sims_all):
    """sims_all: (B, B) with sims[i cap, j img]; returns scalar loss."""
    scores = sims_all.T.astype(np.float64)  # scores[img j, cap i]
    diag = np.diag(scores).copy()
    cost_s = np.maximum(MARGIN + scores - diag[:, None], 0.0)
    cost_im = np.maximum(MARGIN + scores - diag[None, :], 0.0)
    np.fill_diagonal(cost_s, 0.0)
    np.fill_diagonal(cost_im, 0.0)
    return np.array(cost_s.sum() + cost_im.sum(), dtype=np.float32)


def kernel(im, s, s_l, x, _trace=False, _mm_dtype=None, _cfg=None):
    from concourse.bass_utils import run_bass_kernel_spmd

    cfg = _cfg or CONFIG
    n = np.asarray(s_l).astype(np.int64)
    in_maps, caps_of_core, wt_tiles = make_core_inputs(im, s, s_l, cfg)
    nc = get_nc(wt_tiles, cfg)
    res = run_bass_kernel_spmd(nc, in_maps, list(range(N_CORES)), trace=_trace)
    sims_all = np.zeros((B, B), dtype=np.float64)
    for c in range(N_CORES):
        blk = np.asarray(res.results[c]["out"]).astype(np.float64)
        # [p, t, j] -> packed-word-major [t*128+p, j]
        mm = blk.reshape(128, wt_tiles, B).transpose(1, 0, 2).reshape(-1, B)
        off = 0
        for cap in caps_of_core[c]:
            ni = int(n[cap])
            sims_all[cap] = mm[off:off + ni].sum(axis=0) / float(ni)
            off += ni
    loss = loss_from_sims(sims_all)
    if _trace:
        return loss, res
    return loss
